# revision 21
# baseline (speedup 1.0000x reference)
"""Trainium2 Bass kernel for nn_DiffMPC2 (100-step diagonal-QP SGD recursion).

The reference iterates  u <- u - LR*(2*q*u + p)  100 times, i.e. the affine
per-element map  u <- a*u + b  with  a = 1 - 0.02*q,  b = -0.01*p.  Closed
form:  u_100 = a^100 * u0 + b * S_100,  S_100 = sum_{k<100} a^k.

Per element (f32), engines in brackets:
    L  = Ln(1 - 0.02*q)                 [ACT]
    P  = Exp(100*L)  = a^100            [ACT]
    G  = Ln(2*q)                        [ACT]
    R  = Exp(-G)     = 0.5/q            [ACT]
    E  = 1 - P                          [ACT]  (Identity, scale=-1 bias=1)
    Sl = E*R         = 0.01*S_100       [DVE]  (exact unless q small)
    St = 1 - 0.99*q + 0.6468*q^2        [GPSIMD] (Taylor of 0.01*S_100,
                                         exact for small q where 1-P
                                         suffers f32 cancellation)
    S' = q < QHI ? St : Sl              [DVE copy_predicated]
    u  = P*u0 - S'*p                    [DVE]

Sharding: pure data parallel, batch split across 8 cores.  Each core gets
131072 rows x 4 ctrl cols = 524288 elems laid out as [128, 4096] f32.
Only Q[:,12:], p[:,12:], u_init are touched (x_init is dead): 8 MB of HBM
traffic per core, which is the memory roofline for this problem.  The three
inputs are host-packed into one DRAM tensor ([q | p | u0] per partition)
so each chunk needs a single input DMA.

Written in raw bass (explicit per-engine programs + semaphores): the
container's walrus build only allows ONE sync-wait per compute instruction,
which the Tile scheduler's automatic sem assignment keeps exceeding.  With
raw bass every wait is its own instruction.  Double-buffered over
N_CHUNKS column chunks: DMA of chunk c+1 overlaps compute of chunk c.
"""

import sys

for _p in (
    "/root/.axon_site",
    "/root/.axon_site/_ro/trn_rl_repo",
    "/root/.axon_site/_ro/pypackages",
):
    if _p not in sys.path:
        sys.path.append(_p)

import numpy as np

from concourse import bass, mybir
from concourse.bass_utils import run_bass_kernel_spmd

N_CORES = 8
B = 1048576
S_DIM = 12
C_DIM = 4
PARTS = 128
F_TOTAL = (B // N_CORES) * C_DIM // PARTS  # 4096
F_CHUNK = 1024
N_CHUNKS = F_TOTAL // F_CHUNK
NSLOT = 2  # double buffering

QHI = 0.04  # Taylor/LUT branch point

_nc_cache = None


def _build_bass():
    f32 = mybir.dt.float32
    u8 = mybir.dt.uint8
    Alu = mybir.AluOpType
    Act = mybir.ActivationFunctionType

    nc = bass.Bass()

    # Packed input: per partition [q | p | u0], each F_TOTAL wide.
    xin = nc.declare_dram_parameter("xin", [PARTS, 3 * F_TOTAL], f32, isOutput=False)
    uo = nc.declare_dram_parameter("uo", [PARTS, F_TOTAL], f32, isOutput=True)
    xr = xin.ap().rearrange("p (j f) -> p j f", j=3)

    def sb(name, cols, dtype=f32):
        return nc.alloc_sbuf_tensor(name, [PARTS, cols], dtype).ap()

    # Double-buffered tiles (cross-engine handoffs).
    tx = [
        sb(f"tx{s}", 3 * F_CHUNK).rearrange("p (j f) -> p j f", j=3)
        for s in range(NSLOT)
    ]
    tP = [sb(f"tP{s}", F_CHUNK) for s in range(NSLOT)]
    tR = [sb(f"tR{s}", F_CHUNK) for s in range(NSLOT)]
    tE = [sb(f"tE{s}", F_CHUNK) for s in range(NSLOT)]
    tm = [sb(f"tm{s}", F_CHUNK, u8) for s in range(NSLOT)]
    tSt = [sb(f"tSt{s}", F_CHUNK) for s in range(NSLOT)]
    # Engine-local scratch (in-order reuse is safe).
    tL = sb("tL", F_CHUNK)
    tG = sb("tG", F_CHUNK)
    th = sb("th", F_CHUNK)
    th2 = sb("th2", F_CHUNK)
    tS = sb("tS", F_CHUNK)
    tr2 = sb("tr2", F_CHUNK)
    tr1 = sb("tr1", F_CHUNK)
    tout = sb("tout", F_TOTAL)

    with (
        nc.Block() as block,
        nc.semaphore("s_in") as s_in,
        nc.semaphore("s_act") as s_act,
        nc.semaphore("s_tay") as s_tay,
        nc.semaphore("s_dve") as s_dve,
        nc.semaphore("s_out") as s_out,
    ):

        @block.sync
        def _(sp):
            for c in range(N_CHUNKS):
                if c >= NSLOT:
                    # tx slot reuse: all consumers of chunk c-NSLOT done.
                    sp.wait_ge(s_act, c - NSLOT + 1)
                    sp.wait_ge(s_tay, c - NSLOT + 1)
                    sp.wait_ge(s_dve, c - NSLOT + 1)
                sp.dma_start(
                    out=tx[c % NSLOT],
                    in_=xr[:, :, c * F_CHUNK : (c + 1) * F_CHUNK],
                ).then_inc(s_in, 16)
            for c in range(N_CHUNKS):
                sp.wait_ge(s_dve, c + 1)
                sp.dma_start(
                    out=uo.ap()[:, c * F_CHUNK : (c + 1) * F_CHUNK],
                    in_=tout[:, c * F_CHUNK : (c + 1) * F_CHUNK],
                ).then_inc(s_out, 16)
            sp.wait_ge(s_out, 16 * N_CHUNKS)

        @block.scalar
        def _(act):
            for c in range(N_CHUNKS):
                s = c % NSLOT
                tq = tx[s][:, 0, :]
                act.wait_ge(s_in, 16 * (c + 1))
                if c >= NSLOT:
                    # tP/tR/tE slot reuse: DVE chunk c-NSLOT must be done.
                    act.wait_ge(s_dve, c - NSLOT + 1)
                act.activation(tL, tq, Act.Ln, bias=1.0, scale=-0.02)
                act.activation(tP[s], tL, Act.Exp, bias=0.0, scale=100.0)
                act.activation(tG, tq, Act.Ln, bias=0.0, scale=2.0)
                act.activation(tR[s], tG, Act.Exp, bias=0.0, scale=-1.0)
                act.activation(
                    tE[s], tP[s], Act.Identity, bias=1.0, scale=-1.0
                ).then_inc(s_act, 1)

        @block.gpsimd
        def _(g):
            for c in range(N_CHUNKS):
                s = c % NSLOT
                tq = tx[s][:, 0, :]
                g.wait_ge(s_in, 16 * (c + 1))
                if c >= NSLOT:
                    # tm/tSt slot reuse: DVE chunk c-NSLOT consumed them.
                    g.wait_ge(s_dve, c - NSLOT + 1)
                g.tensor_scalar(tm[s], tq, QHI, None, Alu.is_lt)
                g.tensor_scalar(th, tq, 0.6468, -0.99, Alu.mult, Alu.add)
                g.tensor_tensor(th2, th, tq, Alu.mult)
                g.tensor_scalar_add(tSt[s], th2, 1.0).then_inc(s_tay, 1)

        @block.vector
        def _(v):
            for c in range(N_CHUNKS):
                s = c % NSLOT
                tp_ = tx[s][:, 1, :]
                tu = tx[s][:, 2, :]
                sl = slice(c * F_CHUNK, (c + 1) * F_CHUNK)
                v.wait_ge(s_act, c + 1)
                v.tensor_mul(tS, tE[s], tR[s])
                v.wait_ge(s_tay, c + 1)
                v.copy_predicated(tS, tm[s], tSt[s])
                v.tensor_mul(tr2, tS, tp_)
                v.tensor_mul(tr1, tP[s], tu)
                v.tensor_tensor(tout[:, sl], tr1, tr2, Alu.subtract).then_inc(
                    s_dve, 1
                )

    return nc


def _get_nc():
    global _nc_cache
    if _nc_cache is None:
        _nc_cache = _build_bass()
    return _nc_cache


def _prep_in_maps(Q, p, u_init):
    q_u = np.ascontiguousarray(Q[:, S_DIM:], dtype=np.float32).reshape(
        N_CORES, PARTS, F_TOTAL
    )
    p_u = np.ascontiguousarray(p[:, S_DIM:], dtype=np.float32).reshape(
        N_CORES, PARTS, F_TOTAL
    )
    u0 = np.ascontiguousarray(u_init, dtype=np.float32).reshape(
        N_CORES, PARTS, F_TOTAL
    )
    xin = np.concatenate([q_u, p_u, u0], axis=2)  # [8, 128, 3*F_TOTAL]
    return [{"xin": xin[c]} for c in range(N_CORES)]


def kernel(x_init, Q, p, u_init):
    assert Q.shape == (B, S_DIM + C_DIM) and u_init.shape == (B, C_DIM)
    nc = _get_nc()
    in_maps = _prep_in_maps(Q, p, u_init)
    res = run_bass_kernel_spmd(nc, in_maps, list(range(N_CORES)))
    out = np.stack([res.results[c]["uo"] for c in range(N_CORES)])
    return out.reshape(B, C_DIM)


# revision 22
# speedup vs baseline: 2.6285x; 2.6285x over previous
"""Trainium2 Bass kernel for nn_DiffMPC2 (100-step diagonal-QP SGD recursion).

The reference iterates  u <- u - LR*(2*q*u + p)  100 times, i.e. the affine
per-element map  u <- a*u + b  with  a = 1 - 0.02*q,  b = -0.01*p.  Closed
form:  u_100 = a^100 * u0 + b * S_100,  S_100 = sum_{k<100} a^k.

Per element (f32), engines in brackets:
    L   = Ln(1 - 0.02*q)                [ACT]
    P   = Exp(100*L)  = a^100           [ACT]
    G   = Ln(2*q)                       [ACT]
    R   = Exp(-G)     = 0.5/q           [ACT]
    Sq  = Square(sqrt(.6468)*q - .6155) [ACT]  (= .6468q^2 - .99q + .3788)
    St  = -Sq - .6212                   [DVE tensor_scalar]
        = -1 + .99q - .6468q^2            (Taylor of -0.01*S_100; exact for
                                           small q where P-1 cancels in f32)
    Sl  = (P - 1)*R   = -0.01*S_100     [DVE scalar_tensor_tensor]
    S'  = q < QHI ? St : Sl             [DVE copy_predicated]
    u   = P*u0 + S'*p                   [DVE x3]

Sharding: pure data parallel, batch split across 8 cores.  Each core gets
131072 rows x 4 ctrl cols = 524288 elems laid out as [128, 4096] f32.
Only Q[:,12:], p[:,12:], u_init are touched (x_init is dead): 8 MB of HBM
traffic per core, which is the memory roofline for this problem.  The three
inputs are host-packed into one DRAM tensor ([q | p | u0] per partition)
so each chunk needs a single input DMA.

Written in raw bass (explicit per-engine programs + semaphores): the
container's walrus build only allows ONE sync-wait per compute instruction,
which the Tile scheduler's automatic sem assignment keeps exceeding.  With
raw bass every wait is its own instruction.  Double-buffered over
N_CHUNKS column chunks: DMA of chunk c+1 overlaps compute of chunk c.
GPSIMD is intentionally unused: it shares SBUF ports with the DVE and
running elementwise ops there stalls both engines.
"""

import sys

for _p in (
    "/root/.axon_site",
    "/root/.axon_site/_ro/trn_rl_repo",
    "/root/.axon_site/_ro/pypackages",
):
    if _p not in sys.path:
        sys.path.append(_p)

import numpy as np

from concourse import bass, mybir
from concourse.bass_utils import run_bass_kernel_spmd

N_CORES = 8
B = 1048576
S_DIM = 12
C_DIM = 4
PARTS = 128
F_TOTAL = (B // N_CORES) * C_DIM // PARTS  # 4096
F_CHUNK = 1024
N_CHUNKS = F_TOTAL // F_CHUNK
NSLOT = 2  # double buffering

QHI = 0.04  # Taylor/LUT branch point
SQ_SCALE = 0.8042387962341309  # sqrt(0.6468)
SQ_BIAS = -0.6154888272285461  # -0.99 / (2*sqrt(0.6468))
ST_BIAS = -0.6211734414100647  # -(1 - SQ_BIAS^2)

_nc_cache = None


def _build_bass():
    f32 = mybir.dt.float32
    u8 = mybir.dt.uint8
    Alu = mybir.AluOpType
    Act = mybir.ActivationFunctionType

    nc = bass.Bass()

    # Register activation-bias constants (Bass only pre-registers 0/1).
    for val in (SQ_BIAS,):
        t = nc.alloc_sbuf_tensor(f"const-f32-{val}", [128, 1], f32)
        nc.gpsimd.memset(t.ap(), val)
        nc.const_aps.aps[(f32, val)] = t.ap()
    nc.all_engine_barrier()

    # Packed input: per partition [q | p | u0], each F_TOTAL wide.
    xin = nc.declare_dram_parameter("xin", [PARTS, 3 * F_TOTAL], f32, isOutput=False)
    uo = nc.declare_dram_parameter("uo", [PARTS, F_TOTAL], f32, isOutput=True)
    xr = xin.ap().rearrange("p (j f) -> p j f", j=3)

    def sb(name, cols, dtype=f32):
        return nc.alloc_sbuf_tensor(name, [PARTS, cols], dtype).ap()

    # Double-buffered tiles (cross-engine handoffs).
    tx = [
        sb(f"tx{s}", 3 * F_CHUNK).rearrange("p (j f) -> p j f", j=3)
        for s in range(NSLOT)
    ]
    tP = [sb(f"tP{s}", F_CHUNK) for s in range(NSLOT)]
    tR = [sb(f"tR{s}", F_CHUNK) for s in range(NSLOT)]
    tSq = [sb(f"tSq{s}", F_CHUNK) for s in range(NSLOT)]
    # Engine-local scratch (in-order reuse is safe).
    tL = sb("tL", F_CHUNK)
    tG = sb("tG", F_CHUNK)
    tm = sb("tm", F_CHUNK, u8)
    tSt = sb("tSt", F_CHUNK)
    tS = sb("tS", F_CHUNK)
    tr2 = sb("tr2", F_CHUNK)
    tr1 = sb("tr1", F_CHUNK)
    tout = sb("tout", F_TOTAL)

    with (
        nc.Block() as block,
        nc.semaphore("s_in") as s_in,
        nc.semaphore("s_act") as s_act,
        nc.semaphore("s_dve") as s_dve,
        nc.semaphore("s_out") as s_out,
    ):

        @block.sync
        def _(sp):
            for c in range(N_CHUNKS):
                if c >= NSLOT:
                    # tx slot reuse: all consumers of chunk c-NSLOT done.
                    sp.wait_ge(s_act, c - NSLOT + 1)
                    sp.wait_ge(s_dve, c - NSLOT + 1)
                sp.dma_start(
                    out=tx[c % NSLOT],
                    in_=xr[:, :, c * F_CHUNK : (c + 1) * F_CHUNK],
                ).then_inc(s_in, 16)
            for c in range(N_CHUNKS):
                sp.wait_ge(s_dve, c + 1)
                sp.dma_start(
                    out=uo.ap()[:, c * F_CHUNK : (c + 1) * F_CHUNK],
                    in_=tout[:, c * F_CHUNK : (c + 1) * F_CHUNK],
                ).then_inc(s_out, 16)
            sp.wait_ge(s_out, 16 * N_CHUNKS)

        @block.scalar
        def _(act):
            for c in range(N_CHUNKS):
                s = c % NSLOT
                tq = tx[s][:, 0, :]
                act.wait_ge(s_in, 16 * (c + 1))
                if c >= NSLOT:
                    # tP/tR/tSq slot reuse: DVE chunk c-NSLOT must be done.
                    act.wait_ge(s_dve, c - NSLOT + 1)
                act.activation(tL, tq, Act.Ln, bias=1.0, scale=-0.02)
                act.activation(tP[s], tL, Act.Exp, bias=0.0, scale=100.0)
                act.activation(tG, tq, Act.Ln, bias=0.0, scale=2.0)
                act.activation(tR[s], tG, Act.Exp, bias=0.0, scale=-1.0)
                act.activation(
                    tSq[s], tq, Act.Square, bias=SQ_BIAS, scale=SQ_SCALE
                ).then_inc(s_act, 1)

        @block.vector
        def _(v):
            for c in range(N_CHUNKS):
                s = c % NSLOT
                tq = tx[s][:, 0, :]
                tp_ = tx[s][:, 1, :]
                tu = tx[s][:, 2, :]
                sl = slice(c * F_CHUNK, (c + 1) * F_CHUNK)
                v.wait_ge(s_in, 16 * (c + 1))
                v.tensor_scalar(tm, tq, QHI, None, Alu.is_lt)
                v.wait_ge(s_act, c + 1)
                # St = -Sq + ST_BIAS = -1 + 0.99q - 0.6468q^2
                v.tensor_scalar(tSt, tSq[s], -1.0, ST_BIAS, Alu.mult, Alu.add)
                # Sl = (P - 1) * R = -0.01 * S_100
                v.scalar_tensor_tensor(
                    tS, tP[s], 1.0, tR[s], Alu.subtract, Alu.mult
                )
                v.copy_predicated(tS, tm, tSt)
                v.tensor_mul(tr2, tS, tp_)
                v.tensor_mul(tr1, tP[s], tu)
                v.tensor_add(tout[:, sl], tr1, tr2).then_inc(s_dve, 1)

    return nc


def _get_nc():
    global _nc_cache
    if _nc_cache is None:
        _nc_cache = _build_bass()
    return _nc_cache


def _prep_in_maps(Q, p, u_init):
    q_u = np.ascontiguousarray(Q[:, S_DIM:], dtype=np.float32).reshape(
        N_CORES, PARTS, F_TOTAL
    )
    p_u = np.ascontiguousarray(p[:, S_DIM:], dtype=np.float32).reshape(
        N_CORES, PARTS, F_TOTAL
    )
    u0 = np.ascontiguousarray(u_init, dtype=np.float32).reshape(
        N_CORES, PARTS, F_TOTAL
    )
    xin = np.concatenate([q_u, p_u, u0], axis=2)  # [8, 128, 3*F_TOTAL]
    return [{"xin": xin[c]} for c in range(N_CORES)]


def kernel(x_init, Q, p, u_init):
    assert Q.shape == (B, S_DIM + C_DIM) and u_init.shape == (B, C_DIM)
    nc = _get_nc()
    in_maps = _prep_in_maps(Q, p, u_init)
    res = run_bass_kernel_spmd(nc, in_maps, list(range(N_CORES)))
    out = np.stack([res.results[c]["uo"] for c in range(N_CORES)])
    return out.reshape(B, C_DIM)


# revision 24
# speedup vs baseline: 3.3247x; 1.2649x over previous
"""Trainium2 Bass kernel for nn_DiffMPC2 (100-step diagonal-QP SGD recursion).

The reference iterates  u <- u - LR*(2*q*u + p)  100 times, i.e. the affine
per-element map  u <- a*u + b  with  a = 1 - 0.02*q,  b = -0.01*p.  Closed
form:  u_100 = a^100 * u0 + b * S_100,  S_100 = sum_{k<100} a^k.

Per element (f32), engines in brackets:
    L   = Ln(1 - 0.02*q)                [ACT]
    P   = Exp(100*L)  = a^100           [ACT]
    G   = Ln(2*q)                       [ACT]
    R   = Exp(-G)     = 0.5/q           [ACT]
    Sq  = Square(sqrt(.6468)*q - .6155) [ACT]  (= .6468q^2 - .99q + .3788)
    St  = -Sq - .6212                   [DVE tensor_scalar]
        = -1 + .99q - .6468q^2            (Taylor of -0.01*S_100; exact for
                                           small q where P-1 cancels in f32)
    Sl  = (P - 1)*R   = -0.01*S_100     [DVE scalar_tensor_tensor]
    S'  = q < QHI ? St : Sl             [DVE copy_predicated]
    u   = P*u0 + S'*p                   [DVE x3]

Sharding: pure data parallel, batch split across 8 cores.  Each core gets
131072 rows x 4 ctrl cols = 524288 elems laid out as [128, 4096] f32.
Only Q[:,12:], p[:,12:], u_init are touched (x_init is dead): 8 MB of HBM
traffic per core, which is the memory roofline for this problem.  The three
inputs are host-packed into one DRAM tensor ([q | p | u0] per partition)
so each chunk needs a single input DMA.

Written in raw bass (explicit per-engine programs + semaphores): the
container's walrus build only allows ONE sync-wait per compute instruction,
which the Tile scheduler's automatic sem assignment keeps exceeding.  With
raw bass every wait is its own instruction.  Double-buffered over
N_CHUNKS column chunks: DMA of chunk c+1 overlaps compute of chunk c.
GPSIMD is intentionally unused: it shares SBUF ports with the DVE and
running elementwise ops there stalls both engines.
"""

import sys

for _p in (
    "/root/.axon_site",
    "/root/.axon_site/_ro/trn_rl_repo",
    "/root/.axon_site/_ro/pypackages",
):
    if _p not in sys.path:
        sys.path.append(_p)

import numpy as np

from concourse import bass, mybir
from concourse.bass_utils import run_bass_kernel_spmd

N_CORES = 8
B = 1048576
S_DIM = 12
C_DIM = 4
PARTS = 128
F_TOTAL = (B // N_CORES) * C_DIM // PARTS  # 4096
# Small first/last chunks shrink pipeline fill and drain; middle chunks
# amortize per-instruction overhead.
CHUNKS = [512, 1024, 1024, 1024, 512]
assert sum(CHUNKS) == F_TOTAL
N_CHUNKS = len(CHUNKS)
OFFS = [sum(CHUNKS[:i]) for i in range(N_CHUNKS)]
F_MAX = max(CHUNKS)
NSLOT = 3  # ACT->DVE handoff buffering

QHI = 0.04  # Taylor/LUT branch point
SQ_SCALE = 0.8042387962341309  # sqrt(0.6468)
SQ_BIAS = -0.6154888272285461  # -0.99 / (2*sqrt(0.6468))
ST_BIAS = -0.6211734414100647  # -(1 - SQ_BIAS^2)

_nc_cache = None


def _build_bass():
    f32 = mybir.dt.float32
    u8 = mybir.dt.uint8
    Alu = mybir.AluOpType
    Act = mybir.ActivationFunctionType

    nc = bass.Bass()

    # Register activation-bias constants (Bass only pre-registers 0/1).
    for val in (SQ_BIAS,):
        t = nc.alloc_sbuf_tensor(f"const-f32-{val}", [128, 1], f32)
        nc.gpsimd.memset(t.ap(), val)
        nc.const_aps.aps[(f32, val)] = t.ap()
    nc.all_engine_barrier()

    # Packed input: per partition [q | p | u0], each F_TOTAL wide.
    xin = nc.declare_dram_parameter("xin", [PARTS, 3 * F_TOTAL], f32, isOutput=False)
    uo = nc.declare_dram_parameter("uo", [PARTS, F_TOTAL], f32, isOutput=True)
    xr = xin.ap().rearrange("p (j f) -> p j f", j=3)

    def sb(name, cols, dtype=f32):
        return nc.alloc_sbuf_tensor(name, [PARTS, cols], dtype).ap()

    # Input tiles: one slot per chunk -- no reuse, so every input DMA can be
    # issued immediately with no compute-gating.
    tx = [
        sb(f"tx{c}", 3 * CHUNKS[c]).rearrange("p (j f) -> p j f", j=3)
        for c in range(N_CHUNKS)
    ]
    tP = [sb(f"tP{s}", F_MAX) for s in range(NSLOT)]
    tR = [sb(f"tR{s}", F_MAX) for s in range(NSLOT)]
    tSq = [sb(f"tSq{s}", F_MAX) for s in range(NSLOT)]
    # Engine-local scratch (in-order reuse is safe).
    tL = sb("tL", F_MAX)
    tG = sb("tG", F_MAX)
    tm = sb("tm", F_MAX, u8)
    tSt = sb("tSt", F_MAX)
    tS = sb("tS", F_MAX)
    tr2 = sb("tr2", F_MAX)
    tr1 = sb("tr1", F_MAX)
    tout = sb("tout", F_TOTAL)

    with (
        nc.Block() as block,
        nc.semaphore("s_in") as s_in,
        nc.semaphore("s_act") as s_act,
        nc.semaphore("s_dve") as s_dve,
        nc.semaphore("s_out") as s_out,
    ):

        @block.sync
        def _(sp):
            for c in range(N_CHUNKS):
                sp.dma_start(
                    out=tx[c],
                    in_=xr[:, :, OFFS[c] : OFFS[c] + CHUNKS[c]],
                ).then_inc(s_in, 16)
            for c in range(N_CHUNKS):
                sp.wait_ge(s_dve, c + 1)
                sp.dma_start(
                    out=uo.ap()[:, OFFS[c] : OFFS[c] + CHUNKS[c]],
                    in_=tout[:, OFFS[c] : OFFS[c] + CHUNKS[c]],
                ).then_inc(s_out, 16)
            sp.wait_ge(s_out, 16 * N_CHUNKS)

        @block.scalar
        def _(act):
            for c in range(N_CHUNKS):
                s = c % NSLOT
                w = CHUNKS[c]
                tq = tx[c][:, 0, :]
                act.wait_ge(s_in, 16 * (c + 1))
                if c >= NSLOT:
                    # tP/tR/tSq slot reuse: DVE chunk c-NSLOT must be done.
                    act.wait_ge(s_dve, c - NSLOT + 1)
                act.activation(tL[:, :w], tq, Act.Ln, bias=1.0, scale=-0.02)
                act.activation(tP[s][:, :w], tL[:, :w], Act.Exp, bias=0.0, scale=100.0)
                act.activation(tG[:, :w], tq, Act.Ln, bias=0.0, scale=2.0)
                act.activation(tR[s][:, :w], tG[:, :w], Act.Exp, bias=0.0, scale=-1.0)
                act.activation(
                    tSq[s][:, :w], tq, Act.Square, bias=SQ_BIAS, scale=SQ_SCALE
                ).then_inc(s_act, 1)

        @block.vector
        def _(v):
            for c in range(N_CHUNKS):
                s = c % NSLOT
                w = CHUNKS[c]
                tq = tx[c][:, 0, :]
                tp_ = tx[c][:, 1, :]
                tu = tx[c][:, 2, :]
                sl = slice(OFFS[c], OFFS[c] + w)
                v.wait_ge(s_in, 16 * (c + 1))
                v.tensor_scalar(tm[:, :w], tq, QHI, None, Alu.is_lt)
                v.wait_ge(s_act, c + 1)
                # St = -Sq + ST_BIAS = -1 + 0.99q - 0.6468q^2
                v.tensor_scalar(
                    tSt[:, :w], tSq[s][:, :w], -1.0, ST_BIAS, Alu.mult, Alu.add
                )
                # Sl = (P - 1) * R = -0.01 * S_100
                v.scalar_tensor_tensor(
                    tS[:, :w], tP[s][:, :w], 1.0, tR[s][:, :w], Alu.subtract, Alu.mult
                )
                v.copy_predicated(tS[:, :w], tm[:, :w], tSt[:, :w])
                v.tensor_mul(tr2[:, :w], tS[:, :w], tp_)
                v.tensor_mul(tr1[:, :w], tP[s][:, :w], tu)
                v.tensor_add(tout[:, sl], tr1[:, :w], tr2[:, :w]).then_inc(s_dve, 1)

    return nc


def _get_nc():
    global _nc_cache
    if _nc_cache is None:
        _nc_cache = _build_bass()
    return _nc_cache


def _prep_in_maps(Q, p, u_init):
    q_u = np.ascontiguousarray(Q[:, S_DIM:], dtype=np.float32).reshape(
        N_CORES, PARTS, F_TOTAL
    )
    p_u = np.ascontiguousarray(p[:, S_DIM:], dtype=np.float32).reshape(
        N_CORES, PARTS, F_TOTAL
    )
    u0 = np.ascontiguousarray(u_init, dtype=np.float32).reshape(
        N_CORES, PARTS, F_TOTAL
    )
    xin = np.concatenate([q_u, p_u, u0], axis=2)  # [8, 128, 3*F_TOTAL]
    return [{"xin": xin[c]} for c in range(N_CORES)]


def kernel(x_init, Q, p, u_init):
    assert Q.shape == (B, S_DIM + C_DIM) and u_init.shape == (B, C_DIM)
    nc = _get_nc()
    in_maps = _prep_in_maps(Q, p, u_init)
    res = run_bass_kernel_spmd(nc, in_maps, list(range(N_CORES)))
    out = np.stack([res.results[c]["uo"] for c in range(N_CORES)])
    return out.reshape(B, C_DIM)


# revision 29
# speedup vs baseline: 3.4180x; 1.0280x over previous
"""Trainium2 Bass kernel for nn_DiffMPC2 (100-step diagonal-QP SGD recursion).

The reference iterates  u <- u - LR*(2*q*u + p)  100 times, i.e. the affine
per-element map  u <- a*u + b  with  a = 1 - 0.02*q,  b = -0.01*p.  Closed
form:  u_100 = a^100 * u0 + b * S_100,  S_100 = sum_{k<100} a^k.

Per element (f32), engines in brackets:
    L   = Ln(1 - 0.02*q)                [ACT]
    P   = Exp(100*L)  = a^100           [ACT]
    G   = Ln(2*q)                       [ACT]
    R   = Exp(-G)     = 0.5/q           [ACT]
    Sq  = Square(sqrt(.6468)*q - .6155) [ACT]  (= .6468q^2 - .99q + .3788)
    St  = -Sq - .6212                   [DVE tensor_scalar]
        = -1 + .99q - .6468q^2            (Taylor of -0.01*S_100; exact for
                                           small q where P-1 cancels in f32)
    Sl  = (P - 1)*R   = -0.01*S_100     [DVE scalar_tensor_tensor]
    S'  = q < QHI ? St : Sl             [DVE copy_predicated]
    u   = P*u0 + S'*p                   [DVE x3]

Sharding: pure data parallel, batch split across 8 cores.  Each core gets
131072 rows x 4 ctrl cols = 524288 elems laid out as [128, 4096] f32.
Only Q[:,12:], p[:,12:], u_init are touched (x_init is dead): 8 MB of HBM
traffic per core, which is the memory roofline for this problem.  The three
inputs are host-packed into one DRAM tensor ([q | p | u0] per partition)
so each chunk needs a single input DMA.

Written in raw bass (explicit per-engine programs + semaphores): the
container's walrus build only allows ONE sync-wait per compute instruction,
which the Tile scheduler's automatic sem assignment keeps exceeding.  With
raw bass every wait is its own instruction.  Double-buffered over
N_CHUNKS column chunks: DMA of chunk c+1 overlaps compute of chunk c.
GPSIMD is intentionally unused: it shares SBUF ports with the DVE and
running elementwise ops there stalls both engines.
"""

import sys

for _p in (
    "/root/.axon_site",
    "/root/.axon_site/_ro/trn_rl_repo",
    "/root/.axon_site/_ro/pypackages",
):
    if _p not in sys.path:
        sys.path.append(_p)

import numpy as np

from concourse import bass, mybir
from concourse.bass_utils import run_bass_kernel_spmd

N_CORES = 8
B = 1048576
S_DIM = 12
C_DIM = 4
PARTS = 128
F_TOTAL = (B // N_CORES) * C_DIM // PARTS  # 4096
# Small first/last chunks shrink pipeline fill and drain; middle chunks
# amortize per-instruction overhead.
CHUNKS = [256, 512, 768, 1024, 1024, 512]
assert sum(CHUNKS) == F_TOTAL
N_CHUNKS = len(CHUNKS)
OFFS = [sum(CHUNKS[:i]) for i in range(N_CHUNKS)]
F_MAX = max(CHUNKS)
NSLOT = 3  # ACT->DVE handoff buffering

QHI = 0.04  # Taylor/LUT branch point
SQ_SCALE = 0.8042387962341309  # sqrt(0.6468)
SQ_BIAS = -0.6154888272285461  # -0.99 / (2*sqrt(0.6468))
ST_BIAS = -0.6211734414100647  # -(1 - SQ_BIAS^2)

_nc_cache = None


def _build_bass():
    f32 = mybir.dt.float32
    u8 = mybir.dt.uint8
    Alu = mybir.AluOpType
    Act = mybir.ActivationFunctionType

    nc = bass.Bass()

    # Register activation-bias constants (Bass only pre-registers 0/1).
    for val in (SQ_BIAS,):
        t = nc.alloc_sbuf_tensor(f"const-f32-{val}", [128, 1], f32)
        nc.gpsimd.memset(t.ap(), val)
        nc.const_aps.aps[(f32, val)] = t.ap()
    nc.all_engine_barrier()

    # Packed input: per partition [q | p | u0], each F_TOTAL wide.
    xin = nc.declare_dram_parameter("xin", [PARTS, 3 * F_TOTAL], f32, isOutput=False)
    uo = nc.declare_dram_parameter("uo", [PARTS, F_TOTAL], f32, isOutput=True)
    xr = xin.ap().rearrange("p (j f) -> p j f", j=3)

    def sb(name, cols, dtype=f32):
        return nc.alloc_sbuf_tensor(name, [PARTS, cols], dtype).ap()

    # Input tiles: one slot per chunk -- no reuse, so every input DMA can be
    # issued immediately with no compute-gating.
    tx = [
        sb(f"tx{c}", 3 * CHUNKS[c]).rearrange("p (j f) -> p j f", j=3)
        for c in range(N_CHUNKS)
    ]
    tP = [sb(f"tP{s}", F_MAX) for s in range(NSLOT)]
    tR = [sb(f"tR{s}", F_MAX) for s in range(NSLOT)]
    tSq = [sb(f"tSq{s}", F_MAX) for s in range(NSLOT)]
    # Engine-local scratch (in-order reuse is safe).
    tL = sb("tL", F_MAX)
    tG = sb("tG", F_MAX)
    tm = sb("tm", F_MAX, u8)
    tSt = sb("tSt", F_MAX)
    tS = sb("tS", F_MAX)
    tr2 = sb("tr2", F_MAX)
    tr1 = sb("tr1", F_MAX)
    tout = sb("tout", F_TOTAL)

    with (
        nc.Block() as block,
        nc.semaphore("s_in") as s_in,
        nc.semaphore("s_acta") as s_acta,
        nc.semaphore("s_actb") as s_actb,
        nc.semaphore("s_dve") as s_dve,
        nc.semaphore("s_out") as s_out,
    ):

        @block.sync
        def _(sp):
            for c in range(N_CHUNKS):
                sp.dma_start(
                    out=tx[c],
                    in_=xr[:, :, OFFS[c] : OFFS[c] + CHUNKS[c]],
                ).then_inc(s_in, 16)
            for c in range(N_CHUNKS):
                sp.wait_ge(s_dve, c + 1)
                sp.dma_start(
                    out=uo.ap()[:, OFFS[c] : OFFS[c] + CHUNKS[c]],
                    in_=tout[:, OFFS[c] : OFFS[c] + CHUNKS[c]],
                ).then_inc(s_out, 16)
            sp.wait_ge(s_out, 16 * N_CHUNKS)

        @block.scalar
        def _(act):
            for c in range(N_CHUNKS):
                s = c % NSLOT
                w = CHUNKS[c]
                tq = tx[c][:, 0, :]
                act.wait_ge(s_in, 16 * (c + 1))
                if c >= NSLOT:
                    # tP/tR/tSq slot reuse: DVE chunk c-NSLOT must be done.
                    act.wait_ge(s_dve, c - NSLOT + 1)
                act.activation(tL[:, :w], tq, Act.Ln, bias=1.0, scale=-0.02)
                act.activation(tP[s][:, :w], tL[:, :w], Act.Exp, bias=0.0, scale=100.0)
                act.activation(
                    tSq[s][:, :w], tq, Act.Square, bias=SQ_BIAS, scale=SQ_SCALE
                ).then_inc(s_acta, 1)
                act.activation(tG[:, :w], tq, Act.Ln, bias=0.0, scale=2.0)
                act.activation(
                    tR[s][:, :w], tG[:, :w], Act.Exp, bias=0.0, scale=-1.0
                ).then_inc(s_actb, 1)

        @block.vector
        def _(v):
            for c in range(N_CHUNKS):
                s = c % NSLOT
                w = CHUNKS[c]
                tq = tx[c][:, 0, :]
                tp_ = tx[c][:, 1, :]
                tu = tx[c][:, 2, :]
                sl = slice(OFFS[c], OFFS[c] + w)
                v.wait_ge(s_in, 16 * (c + 1))
                v.tensor_scalar(tm[:, :w], tq, QHI, None, Alu.is_lt)
                v.wait_ge(s_acta, c + 1)
                # St = -Sq + ST_BIAS = -1 + 0.99q - 0.6468q^2
                v.tensor_scalar(
                    tSt[:, :w], tSq[s][:, :w], -1.0, ST_BIAS, Alu.mult, Alu.add
                )
                v.tensor_mul(tr1[:, :w], tP[s][:, :w], tu)
                v.wait_ge(s_actb, c + 1)
                # Sl = (P - 1) * R = -0.01 * S_100
                v.scalar_tensor_tensor(
                    tS[:, :w], tP[s][:, :w], 1.0, tR[s][:, :w], Alu.subtract, Alu.mult
                )
                v.copy_predicated(tS[:, :w], tm[:, :w], tSt[:, :w])
                v.tensor_mul(tr2[:, :w], tS[:, :w], tp_)
                v.tensor_add(tout[:, sl], tr1[:, :w], tr2[:, :w]).then_inc(s_dve, 1)

    return nc


def _get_nc():
    global _nc_cache
    if _nc_cache is None:
        _nc_cache = _build_bass()
    return _nc_cache


def _prep_in_maps(Q, p, u_init):
    q_u = np.ascontiguousarray(Q[:, S_DIM:], dtype=np.float32).reshape(
        N_CORES, PARTS, F_TOTAL
    )
    p_u = np.ascontiguousarray(p[:, S_DIM:], dtype=np.float32).reshape(
        N_CORES, PARTS, F_TOTAL
    )
    u0 = np.ascontiguousarray(u_init, dtype=np.float32).reshape(
        N_CORES, PARTS, F_TOTAL
    )
    xin = np.concatenate([q_u, p_u, u0], axis=2)  # [8, 128, 3*F_TOTAL]
    return [{"xin": xin[c]} for c in range(N_CORES)]


def kernel(x_init, Q, p, u_init):
    assert Q.shape == (B, S_DIM + C_DIM) and u_init.shape == (B, C_DIM)
    nc = _get_nc()
    in_maps = _prep_in_maps(Q, p, u_init)
    res = run_bass_kernel_spmd(nc, in_maps, list(range(N_CORES)))
    out = np.stack([res.results[c]["uo"] for c in range(N_CORES)])
    return out.reshape(B, C_DIM)


# revision 34
# speedup vs baseline: 3.6529x; 1.0687x over previous
"""Trainium2 Bass kernel for nn_DiffMPC2 (100-step diagonal-QP SGD recursion).

The reference iterates  u <- u - LR*(2*q*u + p)  100 times, i.e. the affine
per-element map  u <- a*u + b  with  a = 1 - 0.02*q,  b = -0.01*p.  Closed
form:  u_100 = a^100 * u0 + b * S_100,  S_100 = sum_{k<100} a^k.

Per element (f32), engines in brackets:
    L   = Ln(1 - 0.02*q)                [ACT]
    P   = Exp(100*L)  = a^100           [ACT]
    G   = Ln(2*q)                       [ACT]
    R   = Exp(-G)     = 0.5/q           [ACT]
    Sq  = Square(sqrt(.6468)*q - .6155) [ACT]  (= .6468q^2 - .99q + .3788)
    St  = -Sq - .6212                   [DVE tensor_scalar]
        = -1 + .99q - .6468q^2            (Taylor of -0.01*S_100; exact for
                                           small q where P-1 cancels in f32)
    Sl  = (P - 1)*R   = -0.01*S_100     [DVE scalar_tensor_tensor]
    S'  = q < QHI ? St : Sl             [DVE copy_predicated]
    u   = P*u0 + S'*p                   [DVE x3]

Sharding: pure data parallel, batch split across 8 cores.  Each core gets
131072 rows x 4 ctrl cols = 524288 elems laid out as [128, 4096] f32.
Only Q[:,12:], p[:,12:], u_init are touched (x_init is dead): 8 MB of HBM
traffic per core, which is the memory roofline for this problem.  The three
inputs are host-packed into one DRAM tensor ([q | p | u0] per partition)
so each chunk needs a single input DMA.

Written in raw bass (explicit per-engine programs + semaphores): the
container's walrus build only allows ONE sync-wait per compute instruction,
which the Tile scheduler's automatic sem assignment keeps exceeding.  With
raw bass every wait is its own instruction.  Double-buffered over
N_CHUNKS column chunks: DMA of chunk c+1 overlaps compute of chunk c.
GPSIMD is intentionally unused: it shares SBUF ports with the DVE and
running elementwise ops there stalls both engines.
"""

import sys

for _p in (
    "/root/.axon_site",
    "/root/.axon_site/_ro/trn_rl_repo",
    "/root/.axon_site/_ro/pypackages",
):
    if _p not in sys.path:
        sys.path.append(_p)

import numpy as np

from concourse import bass, mybir
from concourse.bass_utils import run_bass_kernel_spmd

N_CORES = 8
B = 1048576
S_DIM = 12
C_DIM = 4
PARTS = 128
F_TOTAL = (B // N_CORES) * C_DIM // PARTS  # 4096
# Small first/last chunks shrink pipeline fill and drain; middle chunks
# amortize per-instruction overhead.
CHUNKS = [256, 512, 768, 1024, 1024, 512]
assert sum(CHUNKS) == F_TOTAL
N_CHUNKS = len(CHUNKS)
OFFS = [sum(CHUNKS[:i]) for i in range(N_CHUNKS)]
F_MAX = max(CHUNKS)
NSLOT = 3  # ACT->DVE handoff buffering

SQ_SCALE = 0.8042387962341309  # sqrt(0.6468)
SQ_BIAS = -0.6154888272285461  # -0.99 / (2*sqrt(0.6468))
ST_BIAS = -0.6211734414100647  # -(1 - SQ_BIAS^2)
# LUT-branch downshift: Sl = (P-1-EPS)*R.  EPS exceeds the worst-case f32
# noise in P (the rounding of 1-0.02q amplifies x100 through the exponent,
# ~3e-6, plus LUT spline error), so wherever Sl is unreliable it lands
# strictly below the Taylor branch and  S' = max(St, Sl)  picks St.  St
# truncates an alternating series, hence St <= true S' everywhere.
EPS = 1e-5

_nc_cache = None


def _build_bass():
    f32 = mybir.dt.float32
    u8 = mybir.dt.uint8
    Alu = mybir.AluOpType
    Act = mybir.ActivationFunctionType

    nc = bass.Bass()

    # Register activation-bias constants (Bass only pre-registers 0/1).
    for val in (SQ_BIAS,):
        t = nc.alloc_sbuf_tensor(f"const-f32-{val}", [128, 1], f32)
        nc.gpsimd.memset(t.ap(), val)
        nc.const_aps.aps[(f32, val)] = t.ap()
    nc.all_engine_barrier()

    # Packed input: per partition [q | p | u0], each F_TOTAL wide.
    xin = nc.declare_dram_parameter("xin", [PARTS, 3 * F_TOTAL], f32, isOutput=False)
    uo = nc.declare_dram_parameter("uo", [PARTS, F_TOTAL], f32, isOutput=True)
    xr = xin.ap().rearrange("p (j f) -> p j f", j=3)

    def sb(name, cols, dtype=f32):
        return nc.alloc_sbuf_tensor(name, [PARTS, cols], dtype).ap()

    # Input tiles: one slot per chunk -- no reuse, so every input DMA can be
    # issued immediately with no compute-gating.
    tx = [
        sb(f"tx{c}", 3 * CHUNKS[c]).rearrange("p (j f) -> p j f", j=3)
        for c in range(N_CHUNKS)
    ]
    tP = [sb(f"tP{s}", F_MAX) for s in range(NSLOT)]
    tR = [sb(f"tR{s}", F_MAX) for s in range(NSLOT)]
    tSq = [sb(f"tSq{s}", F_MAX) for s in range(NSLOT)]
    # Engine-local scratch (in-order reuse is safe).
    tL = sb("tL", F_MAX)
    tG = sb("tG", F_MAX)
    tSt = sb("tSt", F_MAX)
    tS = sb("tS", F_MAX)
    tMx = sb("tMx", F_MAX)
    tr2 = sb("tr2", F_MAX)
    tr1 = sb("tr1", F_MAX)
    tout = sb("tout", F_TOTAL)

    with (
        nc.Block() as block,
        nc.semaphore("s_in") as s_in,
        nc.semaphore("s_acta") as s_acta,
        nc.semaphore("s_actb") as s_actb,
        nc.semaphore("s_dve") as s_dve,
        nc.semaphore("s_out") as s_out,
    ):

        @block.sync
        def _(sp):
            for c in range(N_CHUNKS):
                sp.dma_start(
                    out=tx[c],
                    in_=xr[:, :, OFFS[c] : OFFS[c] + CHUNKS[c]],
                ).then_inc(s_in, 16)
            for c in range(N_CHUNKS):
                sp.wait_ge(s_dve, c + 1)
                sp.dma_start(
                    out=uo.ap()[:, OFFS[c] : OFFS[c] + CHUNKS[c]],
                    in_=tout[:, OFFS[c] : OFFS[c] + CHUNKS[c]],
                ).then_inc(s_out, 16)
            sp.wait_ge(s_out, 16 * N_CHUNKS)

        @block.scalar
        def _(act):
            for c in range(N_CHUNKS):
                s = c % NSLOT
                w = CHUNKS[c]
                tq = tx[c][:, 0, :]
                act.wait_ge(s_in, 16 * (c + 1))
                if c >= NSLOT:
                    # tP/tR/tSq slot reuse: DVE chunk c-NSLOT must be done.
                    act.wait_ge(s_dve, c - NSLOT + 1)
                act.activation(tL[:, :w], tq, Act.Ln, bias=1.0, scale=-0.02)
                act.activation(tP[s][:, :w], tL[:, :w], Act.Exp, bias=0.0, scale=100.0)
                act.activation(
                    tSq[s][:, :w], tq, Act.Square, bias=SQ_BIAS, scale=SQ_SCALE
                ).then_inc(s_acta, 1)
                act.activation(tG[:, :w], tq, Act.Ln, bias=0.0, scale=2.0)
                act.activation(
                    tR[s][:, :w], tG[:, :w], Act.Exp, bias=0.0, scale=-1.0
                ).then_inc(s_actb, 1)

        @block.vector
        def _(v):
            for c in range(N_CHUNKS):
                s = c % NSLOT
                w = CHUNKS[c]
                tp_ = tx[c][:, 1, :]
                tu = tx[c][:, 2, :]
                sl = slice(OFFS[c], OFFS[c] + w)
                # s_acta implies this chunk's input DMA completed (ACT
                # waits s_in before computing), so tp_/tu are also safe.
                v.wait_ge(s_acta, c + 1)
                # St = -Sq + ST_BIAS = -1 + 0.99q - 0.6468q^2
                v.tensor_scalar(
                    tSt[:, :w], tSq[s][:, :w], -1.0, ST_BIAS, Alu.mult, Alu.add
                )
                v.tensor_mul(tr1[:, :w], tP[s][:, :w], tu)
                v.wait_ge(s_actb, c + 1)
                # Sl = (P - 1 - EPS) * R  = -0.01*S_100 - EPS*R
                v.scalar_tensor_tensor(
                    tS[:, :w], tP[s][:, :w], 1.0 + EPS, tR[s][:, :w],
                    Alu.subtract, Alu.mult,
                )
                v.tensor_tensor(tMx[:, :w], tS[:, :w], tSt[:, :w], Alu.max)
                v.tensor_mul(tr2[:, :w], tMx[:, :w], tp_)
                v.tensor_add(tout[:, sl], tr1[:, :w], tr2[:, :w]).then_inc(s_dve, 1)

    return nc


def _get_nc():
    global _nc_cache
    if _nc_cache is None:
        _nc_cache = _build_bass()
    return _nc_cache


def _prep_in_maps(Q, p, u_init):
    q_u = np.ascontiguousarray(Q[:, S_DIM:], dtype=np.float32).reshape(
        N_CORES, PARTS, F_TOTAL
    )
    p_u = np.ascontiguousarray(p[:, S_DIM:], dtype=np.float32).reshape(
        N_CORES, PARTS, F_TOTAL
    )
    u0 = np.ascontiguousarray(u_init, dtype=np.float32).reshape(
        N_CORES, PARTS, F_TOTAL
    )
    xin = np.concatenate([q_u, p_u, u0], axis=2)  # [8, 128, 3*F_TOTAL]
    return [{"xin": xin[c]} for c in range(N_CORES)]


def kernel(x_init, Q, p, u_init):
    assert Q.shape == (B, S_DIM + C_DIM) and u_init.shape == (B, C_DIM)
    nc = _get_nc()
    in_maps = _prep_in_maps(Q, p, u_init)
    res = run_bass_kernel_spmd(nc, in_maps, list(range(N_CORES)))
    out = np.stack([res.results[c]["uo"] for c in range(N_CORES)])
    return out.reshape(B, C_DIM)
